# revision 1
# baseline (speedup 1.0000x reference)
"""Fused EmbeddingBag(mean) + Linear kernel for Trainium2, 8-core data-parallel.

Strategy: batch is sharded 8 ways (2048 bags/core). The embedding table gets a
host-appended zero row; invalid (beyond-length) token slots are redirected to it
on-device, so the length-masked sum becomes a plain sum. Per 128-bag tile, one
indirect DMA gathers all 6400 token rows (each partition = one bag's 50
embeddings), a strided-AP vector reduce sums over the 50 slots, and a single
matmul against [W.T; b; null_emb] applies projection, bias, and the
empty-bag null-embedding select in one shot.
"""

import sys

sys.path.insert(0, "/opt/trn_rl_repo")

from contextlib import ExitStack

import numpy as np

import concourse.bass as bass
import concourse.bacc as bacc
import concourse.mybir as mybir
import concourse.tile as tile
from concourse.bass import IndirectOffsetOnAxis
from concourse.masks import make_identity

VOCAB, EMBED, COND = 100000, 64, 256
B, L = 16384, 50
NCORES = 8
BLOC = B // NCORES  # 2048 bags per core
P = 128
NT = BLOC // P  # 16 tiles per core

F32 = mybir.dt.float32
I32 = mybir.dt.int32


def build_nc(g_bufs: int = 3) -> bass.Bass:
    nc = bacc.Bacc("TRN2", target_bir_lowering=False)

    ids = nc.declare_dram_parameter("ids", [BLOC, L + 1], I32, isOutput=False)
    emb = nc.declare_dram_parameter("emb", [VOCAB + 1, EMBED], F32, isOutput=False)
    wext = nc.declare_dram_parameter("wext", [EMBED + 2, COND], F32, isOutput=False)
    out = nc.declare_dram_parameter("out", [BLOC, COND], F32, isOutput=True)

    op = mybir.AluOpType

    with tile.TileContext(nc) as tc, ExitStack() as ctx:
        const = ctx.enter_context(tc.tile_pool(name="const", bufs=1))
        sb = ctx.enter_context(tc.tile_pool(name="sb", bufs=6))
        gp = ctx.enter_context(tc.tile_pool(name="gp", bufs=g_bufs))
        ps = ctx.enter_context(tc.tile_pool(name="ps", bufs=2, space="PSUM"))

        # One-time constants
        idt = const.tile([P, P], F32, tag="idt")
        make_identity(nc, idt[:])
        iota_l = const.tile([P, L], I32, tag="iota")
        nc.gpsimd.iota(out=iota_l[:], pattern=[[1, L]], base=0, channel_multiplier=0)
        bigc = const.tile([P, L], I32, tag="bigc")
        nc.gpsimd.memset(bigc[:], VOCAB)  # index of the all-zero row
        wext_sb = const.tile([EMBED + 2, COND], F32, tag="wext")
        nc.gpsimd.dma_start(out=wext_sb[:], in_=wext[:])

        for t in range(NT):
            rows = slice(t * P, (t + 1) * P)

            ids_t = sb.tile([P, L + 1], I32, tag="ids")
            nc.gpsimd.dma_start(out=ids_t[:], in_=ids[rows, :])

            lenf = sb.tile([P, 1], F32, tag="lenf")
            nc.vector.tensor_copy(out=lenf[:], in_=ids_t[:, L : L + 1])

            # mask[p, l] = l < len[p]; invalid slots -> zero-row index
            mask_t = sb.tile([P, L], I32, tag="mask")
            nc.vector.tensor_scalar(
                out=mask_t[:], in0=iota_l[:], scalar1=lenf[:, :1], scalar2=None,
                op0=op.is_lt,
            )
            idx_t = sb.tile([P, L], I32, tag="idx")
            nc.vector.select(
                out=idx_t[:], mask=mask_t[:], on_true=ids_t[:, 0:L], on_false=bigc[:]
            )

            # Gather all 50 embeddings per bag: partition p gets bag t*128+p.
            g_t = gp.tile([P, L * EMBED], F32, tag="g")
            for l in range(L):
                nc.gpsimd.indirect_dma_start(
                    out=g_t[:, l * EMBED : (l + 1) * EMBED],
                    out_offset=None,
                    in_=emb[:],
                    in_offset=IndirectOffsetOnAxis(ap=idx_t[:, l : l + 1], axis=0),
                )

            # Sum over the 50 slots (strided view [P, e, l], reduce innermost l)
            s_t = sb.tile([P, EMBED], F32, tag="s")
            nc.vector.tensor_reduce(
                out=s_t[:],
                in_=g_t[:].rearrange("p (l e) -> p e l", l=L, e=EMBED),
                axis=mybir.AxisListType.X,
                op=op.add,
            )

            # mean = sum / max(len, 1); flags for bias-vs-null selection
            den = sb.tile([P, 1], F32, tag="den")
            nc.vector.tensor_scalar_max(out=den[:], in0=lenf[:], scalar1=1.0)
            rec = sb.tile([P, 1], F32, tag="rec")
            nc.vector.reciprocal(out=rec[:], in_=den[:])

            tr = sb.tile([P, EMBED + 2], F32, tag="tr")
            nc.vector.tensor_scalar_mul(
                out=tr[:, 0:EMBED], in0=s_t[:], scalar1=rec[:, :1]
            )
            nc.vector.tensor_scalar(
                out=tr[:, EMBED : EMBED + 1], in0=lenf[:], scalar1=0.0, scalar2=None,
                op0=op.is_gt,
            )
            nc.vector.tensor_scalar(
                out=tr[:, EMBED + 1 : EMBED + 2], in0=lenf[:], scalar1=0.0,
                scalar2=None, op0=op.is_le,
            )

            # [P, 66] -> [66, P] so the projection contracts over E on partitions
            pT = ps.tile([EMBED + 2, P], F32, tag="pT", space="PSUM")
            nc.tensor.transpose(out=pT[:], in_=tr[:], identity=idt[:])
            mT = sb.tile([EMBED + 2, P], F32, tag="mT")
            nc.scalar.copy(out=mT[:], in_=pT[:])

            # out[128, 256] = meanT.T @ [W.T; b; null]: proj + bias + null select
            po = ps.tile([P, COND], F32, tag="po", space="PSUM")
            nc.tensor.matmul(out=po[:], lhsT=mT[:], rhs=wext_sb[:], start=True, stop=True)
            ob = sb.tile([P, COND], F32, tag="ob")
            nc.scalar.copy(out=ob[:], in_=po[:])
            nc.gpsimd.dma_start(out=out[rows, :], in_=ob[:])

    nc.compile()
    return nc


_NC_CACHE: dict = {}


def _get_nc(g_bufs: int = 3) -> bass.Bass:
    if g_bufs not in _NC_CACHE:
        _NC_CACHE[g_bufs] = build_nc(g_bufs)
    return _NC_CACHE[g_bufs]


def make_in_maps(token_ids, lengths, emb_table, W, b, null_emb):
    lens32 = np.asarray(lengths).astype(np.int32, copy=False).reshape(B, 1)
    ids32 = np.ascontiguousarray(
        np.concatenate(
            [np.asarray(token_ids).astype(np.int32, copy=False), lens32], axis=1
        )
    )
    emb_ext = np.concatenate(
        [np.asarray(emb_table, dtype=np.float32), np.zeros((1, EMBED), np.float32)]
    )
    wext = np.concatenate(
        [
            np.asarray(W, dtype=np.float32).T,  # [64, 256]
            np.asarray(b, dtype=np.float32)[None, :],
            np.asarray(null_emb, dtype=np.float32)[None, :],
        ]
    )  # [66, 256]
    return [
        {
            "ids": ids32[c * BLOC : (c + 1) * BLOC],
            "emb": emb_ext,
            "wext": wext,
        }
        for c in range(NCORES)
    ]


def kernel(token_ids, lengths, emb_table, W, b, null_emb, **run_kwargs):
    from concourse.bass_utils import run_bass_kernel_spmd

    nc = _get_nc()
    in_maps = make_in_maps(token_ids, lengths, emb_table, W, b, null_emb)
    res = run_bass_kernel_spmd(nc, in_maps, core_ids=list(range(NCORES)), **run_kwargs)
    out = np.concatenate([res.results[c]["out"] for c in range(NCORES)], axis=0)
    return out



# revision 8
# speedup vs baseline: 1.5351x; 1.5351x over previous
"""Fused EmbeddingBag(mean) + Linear kernel for Trainium2, 8-core data-parallel.

Strategy: batch sharded 8 ways (2048 bags/core). The embedding table is
host-packed into bf16 "quad slots" [25002, 256]: slot s>=1 holds vocab rows
4(s-1)..4(s-1)+3, slot 0 is zeros. Token t lives in slot (t>>2)+1 at sub-row
t&3, so slot indices fit int16 — which unlocks the custom InstDMAGatherAnt
ucode (vectorized descriptor generation, ~0.3 ns/desc vs ~1 us fixed cost per
generic indirect DMA). Per 128-bag tile one dma_gather fetches all 6400 quad
slots; a host-built bf16 mask M[p, l, j] = (j == t&3 && l < len) / max(len,1)
then selects the right sub-row AND applies the mean scaling in a single
broadcast multiply; a strided reduce sums over (l, j); and one matmul against
[W.T; b; null_emb] applies projection, bias, and empty-bag select.
"""

import sys

sys.path.insert(0, "/opt/trn_rl_repo")

from contextlib import ExitStack

import numpy as np
import ml_dtypes

import concourse.bass as bass
import concourse.bacc as bacc
import concourse.mybir as mybir
import concourse.tile as tile
from concourse.bass import broadcast_tensor_aps
from concourse.masks import make_identity

VOCAB, EMBED, COND = 100000, 64, 256
B, L = 16384, 50
NCORES = 8
BLOC = B // NCORES  # 2048 bags per core
P = 128
NT = BLOC // P  # 16 tiles per core

NSLOT = VOCAB // 4 + 2  # zero slot + 25000 quad slots
QROW = 4 * EMBED  # 256 bf16 per quad slot
NIDX = P * L  # 6400 gathered slots per tile
NWRAP = NIDX // 16  # idx stream columns per tile
NCJ = L * 4  # (l, sub-row) pairs per bag

F32 = mybir.dt.float32
BF16 = mybir.dt.bfloat16
I16 = mybir.dt.int16

BF16_NP = ml_dtypes.bfloat16


# Gather chunking: the SWDGE descriptor ring fits only ~65-96 descs/lane on
# this firmware (1024 idx = 65 descs/lane works; 1536 = 97 crashes the DMA).
# Split each tile's 6400 indices into 8-slot (1024-idx) chunks and round-robin
# the 4 SWDGE queues so chunk N+1's descriptors generate while chunk N drains.
CHUNK_SLOTS = [8, 8, 8, 8, 8, 8, 2]
NQUEUES = 4


def build_nc() -> bass.Bass:
    nc = bacc.Bacc("TRN2", target_bir_lowering=False, num_swdge_queues=NQUEUES)

    embq = nc.declare_dram_parameter("embq", [NSLOT, QROW], BF16, isOutput=False)
    idxw = nc.declare_dram_parameter("idxw", [P, NT * NWRAP], I16, isOutput=False)
    mw = nc.declare_dram_parameter("mw", [P, NT * NCJ], BF16, isOutput=False)
    fw = nc.declare_dram_parameter("fw", [P, NT * 2], F32, isOutput=False)
    wext = nc.declare_dram_parameter("wext", [EMBED + 2, COND], F32, isOutput=False)
    out = nc.declare_dram_parameter("out", [BLOC, COND], F32, isOutput=True)

    op = mybir.AluOpType

    with tile.TileContext(nc) as tc, ExitStack() as ctx:
        const = ctx.enter_context(tc.tile_pool(name="const", bufs=1))
        sb = ctx.enter_context(tc.tile_pool(name="sb", bufs=6))
        gp = ctx.enter_context(tc.tile_pool(name="gp", bufs=3))
        ps = ctx.enter_context(tc.tile_pool(name="ps", bufs=2, space="PSUM"))

        # One-time constants
        idt = const.tile([P, P], F32, tag="idt")
        make_identity(nc, idt[:])
        idx_sb = const.tile([P, NT * NWRAP], I16, tag="idx")
        nc.sync.dma_start(out=idx_sb[:], in_=idxw[:])
        m_sb = const.tile([P, NT * NCJ], BF16, tag="m")
        nc.sync.dma_start(out=m_sb[:], in_=mw[:])
        f_sb = const.tile([P, NT * 2], F32, tag="f")
        nc.sync.dma_start(out=f_sb[:], in_=fw[:])
        wext_sb = const.tile([EMBED + 2, COND], F32, tag="wext")
        nc.sync.dma_start(out=wext_sb[:], in_=wext[:])

        chunk = 0
        for t in range(NT):
            rows = slice(t * P, (t + 1) * P)

            # Gather this tile's 6400 quad slots in ring-sized chunks.
            gq = gp.tile([P, L * QROW], BF16, tag="gq")
            l0 = 0
            for nsl in CHUNK_SLOTS:
                nidx = nsl * P
                nc.gpsimd.dma_gather(
                    out_ap=gq[:, l0 * QROW : (l0 + nsl) * QROW].rearrange(
                        "p (l e) -> p l e", l=nsl, e=QROW
                    ),
                    in_ap=embq[:],
                    idxs_ap=idx_sb[
                        :, t * NWRAP + l0 * 8 : t * NWRAP + (l0 + nsl) * 8
                    ],
                    num_idxs=nidx,
                    num_idxs_reg=nidx,
                    elem_size=QROW,
                    queue_num=chunk % NQUEUES,
                )
                l0 += nsl
                chunk += 1

            # Select sub-row + length mask + 1/len scaling in one broadcast
            # multiply: gq[p, cj, e] *= M[p, cj] (e broadcast via stride-0).
            g3 = gq[:].rearrange("p (cj e) -> p cj e", cj=NCJ, e=EMBED)
            m3 = m_sb[:, t * NCJ : (t + 1) * NCJ].rearrange(
                "p (cj one) -> p cj one", one=1
            )
            g3b, m3b = broadcast_tensor_aps(g3, m3)
            nc.vector.tensor_mul(out=g3, in0=g3, in1=m3b)

            # mean[p, e] = sum over (l, j); flags for bias-vs-null selection.
            tr = sb.tile([P, EMBED + 2], F32, tag="tr")
            nc.vector.tensor_reduce(
                out=tr[:, 0:EMBED],
                in_=gq[:].rearrange("p (cj e) -> p e cj", cj=NCJ, e=EMBED),
                axis=mybir.AxisListType.X,
                op=op.add,
            )
            nc.vector.tensor_copy(
                out=tr[:, EMBED : EMBED + 2], in_=f_sb[:, 2 * t : 2 * t + 2]
            )

            # [P, 66] -> [66, P] so the projection contracts over E on partitions
            pT = ps.tile([EMBED + 2, P], F32, tag="pT", space="PSUM")
            nc.tensor.transpose(out=pT[:], in_=tr[:], identity=idt[:])
            mT = sb.tile([EMBED + 2, P], F32, tag="mT")
            nc.scalar.copy(out=mT[:], in_=pT[:])

            # out[128, 256] = meanT.T @ [W.T; b; null]: proj + bias + null select
            po = ps.tile([P, COND], F32, tag="po", space="PSUM")
            nc.tensor.matmul(out=po[:], lhsT=mT[:], rhs=wext_sb[:], start=True, stop=True)
            ob = sb.tile([P, COND], F32, tag="ob")
            nc.scalar.copy(out=ob[:], in_=po[:])
            nc.sync.dma_start(out=out[rows, :], in_=ob[:])

    nc.compile()
    return nc


_NC_CACHE: dict = {}


def _get_nc() -> bass.Bass:
    if "nc" not in _NC_CACHE:
        _NC_CACHE["nc"] = build_nc()
    return _NC_CACHE["nc"]


def _pack_embq(emb_table: np.ndarray) -> np.ndarray:
    emb_bf = np.asarray(emb_table, dtype=np.float32).astype(BF16_NP)  # [V, E]
    T = np.zeros((NSLOT, QROW), dtype=BF16_NP)
    T[1 : 1 + VOCAB // 4] = emb_bf.reshape(VOCAB // 4, QROW)
    return T


def make_in_maps(token_ids, lengths, emb_table, W, b, null_emb):
    ids = np.asarray(token_ids).astype(np.int64, copy=False)  # [B, L]
    lens = np.asarray(lengths).astype(np.int64, copy=False)  # [B]

    valid = np.arange(L)[None, :] < lens[:, None]  # [B, L]
    idx16 = np.where(valid, (ids >> 2) + 1, 0).astype(np.int16)  # [B, L]
    rec = (1.0 / np.maximum(lens, 1)).astype(np.float32)  # [B]
    sub = (ids & 3).astype(np.int64)  # [B, L]
    M = (
        (sub[:, :, None] == np.arange(4)[None, None, :]) & valid[:, :, None]
    ).astype(np.float32) * rec[:, None, None]  # [B, L, 4]
    M = M.astype(BF16_NP)
    fz = np.stack([(lens > 0), (lens == 0)], axis=1).astype(np.float32)  # [B, 2]

    embq = _pack_embq(emb_table)
    wext = np.concatenate(
        [
            np.asarray(W, dtype=np.float32).T,  # [64, 256]
            np.asarray(b, dtype=np.float32)[None, :],
            np.asarray(null_emb, dtype=np.float32)[None, :],
        ]
    )  # [66, 256]

    in_maps = []
    for c in range(NCORES):
        sl = slice(c * BLOC, (c + 1) * BLOC)
        # idx stream: token (bag=t*128+p, l) at flat position i = l*128 + p,
        # wrapped into 16 partitions (i%16, i//16), replicated to 128.
        A = idx16[sl].reshape(NT, P, L).transpose(0, 2, 1)  # [NT, L, P]
        flat = A.reshape(NT, NIDX)
        wrap = flat.reshape(NT, NWRAP, 16).transpose(0, 2, 1)  # [NT, 16, NWRAP]
        rep = np.broadcast_to(wrap[:, None], (NT, 8, 16, NWRAP)).reshape(
            NT, P, NWRAP
        )
        idxw = np.ascontiguousarray(
            rep.transpose(1, 0, 2).reshape(P, NT * NWRAP)
        )
        # masks: mw[p, t*200 + l*4 + j]
        Mc = M[sl].reshape(NT, P, NCJ).transpose(1, 0, 2)  # [P, NT, 200]
        mwc = np.ascontiguousarray(Mc.reshape(P, NT * NCJ))
        # flags: fw[p, 2t:2t+2]
        Fc = fz[sl].reshape(NT, P, 2).transpose(1, 0, 2)
        fwc = np.ascontiguousarray(Fc.reshape(P, NT * 2))
        in_maps.append(
            {"embq": embq, "idxw": idxw, "mw": mwc, "fw": fwc, "wext": wext}
        )
    return in_maps


def kernel(token_ids, lengths, emb_table, W, b, null_emb, **run_kwargs):
    from concourse.bass_utils import run_bass_kernel_spmd

    nc = _get_nc()
    in_maps = make_in_maps(token_ids, lengths, emb_table, W, b, null_emb)
    res = run_bass_kernel_spmd(nc, in_maps, core_ids=list(range(NCORES)), **run_kwargs)
    out = np.concatenate([res.results[c]["out"] for c in range(NCORES)], axis=0)
    return out


# revision 9
# speedup vs baseline: 3.2921x; 2.1446x over previous
"""Fused EmbeddingBag(mean) + Linear kernel for Trainium2, 8-core data-parallel.

Strategy: batch sharded 8 ways (2048 bags/core). The embedding table is
host-packed into bf16 "quad slots" [25002, 256]: slot s>=1 holds vocab rows
4(s-1)..4(s-1)+3, slot 0 is zeros. Token t lives in slot (t>>2)+1 at sub-row
t&3, so slot indices fit int16 — which unlocks the custom InstDMAGatherAnt
ucode (vectorized Q7 descriptor generation, ~3 ns/desc vs ~1 us fixed cost
per generic indirect DMA, which only carries one index per partition).

Q7 descriptor generation is the critical path, so bags are host-sorted by
length (descending): tile t then only needs cols_t = ceil(max_len_t/8)*8
token columns, and columns beyond cols_t are never gathered. The per-tile
column counts are baked into the compiled program (cache-keyed; a different
length profile recompiles). cols_t is monotonically non-increasing, so a
recycled gather buffer is always fully covered by what a previous tile wrote
— no stale-SBUF reads.

Per tile: ring-capacity-sized (<=1024 idx, 65 descs/lane) dma_gather chunks
round-robin 4 SWDGE queues; the idle Scalar (ACT) engine expands the
host-built bf16 mask M[p, l, j] = (j == t&3 && l < len) / max(len, 1) across
the 64 embedding lanes; the Vector engine then runs a fully contiguous bf16
multiply (2x mode) and a strided (l, j)-reduce; one matmul against
[W.T; b; null_emb] applies projection, bias, and empty-bag select. The host
un-permutes the output rows.
"""

import sys

sys.path.insert(0, "/opt/trn_rl_repo")

from contextlib import ExitStack

import numpy as np
import ml_dtypes

import concourse.bass as bass
import concourse.bacc as bacc
import concourse.mybir as mybir
import concourse.tile as tile
from concourse.bass import broadcast_tensor_aps
from concourse.masks import make_identity

VOCAB, EMBED, COND = 100000, 64, 256
B, L = 16384, 50
NCORES = 8
BLOC = B // NCORES  # 2048 bags per core
P = 128
NT = BLOC // P  # 16 tiles per core

NSLOT = VOCAB // 4 + 2  # zero slot + 25000 quad slots
QROW = 4 * EMBED  # 256 bf16 per quad slot
CHUNK_COLS = 8  # 1024 idx = 65 descs/lane; ring fits ~65-96

F32 = mybir.dt.float32
BF16 = mybir.dt.bfloat16
I16 = mybir.dt.int16

BF16_NP = ml_dtypes.bfloat16
NQUEUES = 4


def build_nc(cols: tuple) -> bass.Bass:
    """cols[t] = token columns gathered for tile t (multiple of CHUNK_COLS or
    the final partial, non-increasing, cols[t] <= L)."""
    assert len(cols) == NT and all(1 <= c <= L for c in cols)
    tot_cols = sum(cols)
    off = np.concatenate([[0], np.cumsum(cols)])  # column offsets per tile

    nc = bacc.Bacc("TRN2", target_bir_lowering=False, num_swdge_queues=NQUEUES)

    embq = nc.declare_dram_parameter("embq", [NSLOT, QROW], BF16, isOutput=False)
    idxw = nc.declare_dram_parameter("idxw", [P, tot_cols * 8], I16, isOutput=False)
    mw = nc.declare_dram_parameter("mw", [P, tot_cols * 4], BF16, isOutput=False)
    fw = nc.declare_dram_parameter("fw", [P, NT * 2], F32, isOutput=False)
    wext = nc.declare_dram_parameter("wext", [EMBED + 2, COND], F32, isOutput=False)
    out = nc.declare_dram_parameter("out", [BLOC, COND], F32, isOutput=True)

    op = mybir.AluOpType

    with tile.TileContext(nc) as tc, ExitStack() as ctx:
        const = ctx.enter_context(tc.tile_pool(name="const", bufs=1))
        sb = ctx.enter_context(tc.tile_pool(name="sb", bufs=6))
        gp = ctx.enter_context(tc.tile_pool(name="gp", bufs=2))
        mx = ctx.enter_context(tc.tile_pool(name="mx", bufs=2))
        ps = ctx.enter_context(tc.tile_pool(name="ps", bufs=2, space="PSUM"))

        # One-time constants
        idt = const.tile([P, P], F32, tag="idt")
        make_identity(nc, idt[:])
        idx_sb = const.tile([P, tot_cols * 8], I16, tag="idx")
        nc.sync.dma_start(out=idx_sb[:], in_=idxw[:])
        m_sb = const.tile([P, tot_cols * 4], BF16, tag="m")
        nc.sync.dma_start(out=m_sb[:], in_=mw[:])
        f_sb = const.tile([P, NT * 2], F32, tag="f")
        nc.sync.dma_start(out=f_sb[:], in_=fw[:])
        wext_sb = const.tile([EMBED + 2, COND], F32, tag="wext")
        nc.sync.dma_start(out=wext_sb[:], in_=wext[:])

        chunk = 0
        for t in range(NT):
            rows = slice(t * P, (t + 1) * P)
            ct = cols[t]
            ncj = ct * 4

            # Gather this tile's ct*128 quad slots in ring-sized chunks.
            gq = gp.tile([P, L * QROW], BF16, tag="gq")
            l0 = 0
            while l0 < ct:
                nsl = min(CHUNK_COLS, ct - l0)
                nidx = nsl * P
                c0 = off[t] + l0
                nc.gpsimd.dma_gather(
                    out_ap=gq[:, l0 * QROW : (l0 + nsl) * QROW].rearrange(
                        "p (l e) -> p l e", l=nsl, e=QROW
                    ),
                    in_ap=embq[:],
                    idxs_ap=idx_sb[:, c0 * 8 : (c0 + nsl) * 8],
                    num_idxs=nidx,
                    num_idxs_reg=nidx,
                    elem_size=QROW,
                    queue_num=chunk % NQUEUES,
                )
                l0 += nsl
                chunk += 1

            # ACT expands M[p, cj] across the 64 embedding lanes (stride-0
            # broadcast read, contiguous write) so the DVE multiply below
            # stays contiguous and runs in 2x bf16 mode.
            mexp = mx.tile([P, L * QROW], BF16, tag="mexp")
            m3 = m_sb[:, off[t] * 4 : off[t] * 4 + ncj].rearrange(
                "p (cj one) -> p cj one", one=1
            )
            me3 = mexp[:, : ncj * EMBED].rearrange(
                "p (cj e) -> p cj e", cj=ncj, e=EMBED
            )
            _, m3b = broadcast_tensor_aps(me3, m3)
            nc.scalar.copy(out=me3, in_=m3b)

            # Sub-row select + length mask + 1/len scaling: gq *= mexp.
            nc.vector.tensor_mul(
                out=gq[:, : ncj * EMBED],
                in0=gq[:, : ncj * EMBED],
                in1=mexp[:, : ncj * EMBED],
            )

            # mean[p, e] = sum over (l, j); flags for bias-vs-null selection.
            tr = sb.tile([P, EMBED + 2], F32, tag="tr")
            nc.vector.tensor_reduce(
                out=tr[:, 0:EMBED],
                in_=gq[:, : ncj * EMBED].rearrange(
                    "p (cj e) -> p e cj", cj=ncj, e=EMBED
                ),
                axis=mybir.AxisListType.X,
                op=op.add,
            )
            nc.vector.tensor_copy(
                out=tr[:, EMBED : EMBED + 2], in_=f_sb[:, 2 * t : 2 * t + 2]
            )

            # [P, 66] -> [66, P] so the projection contracts over E on partitions
            pT = ps.tile([EMBED + 2, P], F32, tag="pT", space="PSUM")
            nc.tensor.transpose(out=pT[:], in_=tr[:], identity=idt[:])
            mT = sb.tile([EMBED + 2, P], F32, tag="mT")
            nc.scalar.copy(out=mT[:], in_=pT[:])

            # out[128, 256] = meanT.T @ [W.T; b; null]: proj + bias + null select
            po = ps.tile([P, COND], F32, tag="po", space="PSUM")
            nc.tensor.matmul(out=po[:], lhsT=mT[:], rhs=wext_sb[:], start=True, stop=True)
            ob = sb.tile([P, COND], F32, tag="ob")
            nc.scalar.copy(out=ob[:], in_=po[:])
            nc.sync.dma_start(out=out[rows, :], in_=ob[:])

    nc.compile()
    return nc


_NC_CACHE: dict = {}


def _get_nc(cols: tuple) -> bass.Bass:
    if cols not in _NC_CACHE:
        _NC_CACHE[cols] = build_nc(cols)
    return _NC_CACHE[cols]


def _pack_embq(emb_table: np.ndarray) -> np.ndarray:
    emb_bf = np.asarray(emb_table, dtype=np.float32).astype(BF16_NP)  # [V, E]
    T = np.zeros((NSLOT, QROW), dtype=BF16_NP)
    T[1 : 1 + VOCAB // 4] = emb_bf.reshape(VOCAB // 4, QROW)
    return T


def prep(token_ids, lengths, emb_table, W, b, null_emb):
    """Returns (cols, in_maps, perm). Bags are sorted by length (descending)
    within each core; perm maps sorted row -> original row."""
    ids = np.asarray(token_ids).astype(np.int64, copy=False)  # [B, L]
    lens = np.asarray(lengths).astype(np.int64, copy=False)  # [B]

    # Sort bags per core by length descending (stable for determinism).
    perm = np.concatenate(
        [
            c * BLOC + np.argsort(-lens[c * BLOC : (c + 1) * BLOC], kind="stable")
            for c in range(NCORES)
        ]
    )
    ids = ids[perm]
    lens = lens[perm]

    # Per-tile column counts, maxed across cores so one SPMD program fits all.
    lt = lens.reshape(NCORES, NT, P)
    maxlen = lt.max(axis=2).max(axis=0)  # [NT]
    cols = tuple(
        int(min(L, -(-m // CHUNK_COLS) * CHUNK_COLS)) if m > 0 else 1
        for m in np.maximum(maxlen, 1)
    )

    valid = np.arange(L)[None, :] < lens[:, None]  # [B, L]
    idx16 = np.where(valid, (ids >> 2) + 1, 0).astype(np.int16)  # [B, L]
    rec = (1.0 / np.maximum(lens, 1)).astype(np.float32)  # [B]
    sub = (ids & 3).astype(np.int64)  # [B, L]
    M = (
        (sub[:, :, None] == np.arange(4)[None, None, :]) & valid[:, :, None]
    ).astype(np.float32) * rec[:, None, None]  # [B, L, 4]
    M = M.astype(BF16_NP)
    fz = np.stack([(lens > 0), (lens == 0)], axis=1).astype(np.float32)  # [B, 2]

    embq = _pack_embq(emb_table)
    wext = np.concatenate(
        [
            np.asarray(W, dtype=np.float32).T,  # [64, 256]
            np.asarray(b, dtype=np.float32)[None, :],
            np.asarray(null_emb, dtype=np.float32)[None, :],
        ]
    )  # [66, 256]

    in_maps = []
    for c in range(NCORES):
        sl = slice(c * BLOC, (c + 1) * BLOC)
        A = idx16[sl].reshape(NT, P, L)  # [NT, P, L]
        Mc = M[sl].reshape(NT, P, L, 4)
        idx_parts, m_parts = [], []
        for t in range(NT):
            ct = cols[t]
            # idx stream: token (bag=t*128+p, l) at flat position i = l*128+p,
            # wrapped into 16 partitions (i%16, i//16), replicated to 128.
            At = A[t, :, :ct].T  # [ct, P]
            flat = At.reshape(ct * P)
            wrap = flat.reshape(ct * 8, 16).T  # [16, ct*8]
            idx_parts.append(np.tile(wrap, (8, 1)))  # [128, ct*8]
            m_parts.append(Mc[t, :, :ct, :].reshape(P, ct * 4))
        idxw = np.ascontiguousarray(np.concatenate(idx_parts, axis=1))
        mwc = np.ascontiguousarray(np.concatenate(m_parts, axis=1))
        Fc = fz[sl].reshape(NT, P, 2).transpose(1, 0, 2)
        fwc = np.ascontiguousarray(Fc.reshape(P, NT * 2))
        in_maps.append(
            {"embq": embq, "idxw": idxw, "mw": mwc, "fw": fwc, "wext": wext}
        )
    return cols, in_maps, perm


def make_in_maps(token_ids, lengths, emb_table, W, b, null_emb):
    return prep(token_ids, lengths, emb_table, W, b, null_emb)[1]


def kernel(token_ids, lengths, emb_table, W, b, null_emb, **run_kwargs):
    from concourse.bass_utils import run_bass_kernel_spmd

    cols, in_maps, perm = prep(token_ids, lengths, emb_table, W, b, null_emb)
    nc = _get_nc(cols)
    res = run_bass_kernel_spmd(nc, in_maps, core_ids=list(range(NCORES)), **run_kwargs)
    sorted_out = np.concatenate(
        [res.results[c]["out"] for c in range(NCORES)], axis=0
    )
    out = np.empty_like(sorted_out)
    out[perm] = sorted_out
    return out
